# revision 17
# baseline (speedup 1.0000x reference)
"""Trainium2 Bass kernel for nn_DFTModel (segment_reduce), v2.

Math: for each level l in {1,2,3} the reference multiplies channel l-1 by a
block-tiled radial ("square ring") filter and sums nested square rectangles.
Nested-square sums of the raw image Q[r] are computed as
    Q = rowsum(M o (A~ @ X))       (PE matmul with 0/1 interval operator A~,
                                    mask-multiply + axis reduce)
and the filter weights fold into a tiny per-block upper-triangular transform
    c = U @ Q
so the 222x222 filter product is never materialized.

v2 layout/schedule changes vs v1 (which was serialized on the Sync queue's
DMA issue rate at ~94 GB/s effective):
  * x is converted to float16 on the host and stored 2 rows per partition
    (111 partitions x 1332 elems, one contiguous 2664 B descriptor per
    partition) - half the bytes and half the DMA descriptors. The row-parity
    split becomes two accumulating matmuls per unit with parity-split A~
    operators.
  * image DMAs are spread across two queues (sync early, gpsimd late - the
    gpsimd queue sits behind the startup all-core barrier); MLP weights are
    prefetched in fp16 on the vector queue at t=0 and stay SBUF-resident.
  * level 3 is repacked into ONE 111-row unit (row offsets 0/37/74). All
    engine APs start at partition base 0, so no DMA scatter is needed
    anywhere: phase 2 is six stacked-operator matmuls (one per 128-feature
    tile of a padded 768-feature space), and fc1 weights / bn params are
    host-permuted to match.
  * phase-1 masked reduce is split across engines: ACT casts PSUM z to fp16
    in SBUF, mask-multiplies run on DVE (2x fp16 mode) for early groups and
    on GpSimd for late groups, DVE does the axis reduces into fp32 q.
  * the MLP runs feature-major with stationary fp16 weights (out[n, img]),
    biases applied by the ACT engine per partition - no transposes, no ones
    rows except fc5, whose bias rides as an extra weight k-tile against an
    all-ones activation row.
BatchNorm batch statistics are exchanged with one 6 KB AllReduce.
"""

import os
import sys
import types

import numpy as np

for _p in ("/opt/trn_rl_repo",):
    if _p not in sys.path and os.path.isdir(_p):
        sys.path.insert(0, _p)

import concourse.bacc as bacc
import concourse.bass as bass
import concourse.mybir as mybir
import concourse.tile as tile

F32 = mybir.dt.float32
F16 = mybir.dt.float16

IMAGE = 222
HP = 111                      # partitions for the 2-rows-per-partition layout
N_CORES = 8
IMG_PER_CORE = 32
NUM_CLASSES = 1000
FC_DIMS = [768, 2048, 1024, 512, 128, 1000]   # 768 = padded feature count


def _get_dim(level):
    d = IMAGE / level
    if d % 2 == 1:
        d += 1
    return int(d // 2)


MAXR = {l: _get_dim(l) for l in (1, 2, 3)}          # 111, 56, 37
FBLK = {l: IMAGE // l for l in (1, 2, 3)}           # 222, 111, 74
NUM_COEFF = sum(l * l * MAXR[l] for l in (1, 2, 3))  # 668

# units: (name, level, [(bi, row_offset)...], n_rows)
UNITS = [
    ("u1", 1, [(0, 0)], 111),
    ("u2", 2, [(0, 0), (1, 64)], 120),
    ("u3", 3, [(0, 0), (1, 37), (2, 74)], 111),
]

# phase-2 k-tiles of the padded 768-feature space: (q-unit, bj)
KTS = [("u1", 0), ("u2", 0), ("u2", 1), ("u3", 0), ("u3", 1), ("u3", 2)]


def _unit_info(name):
    for nm, lv, pk, nr in UNITS:
        if nm == name:
            return lv, pk, nr
    raise KeyError(name)


def _orig_block_off(l, bi, bj):
    base = {1: 0, 2: 111, 3: 335}[l]
    return base + (bi * l + bj) * MAXR[l]


def _padded_map():
    """perm[kt*128 + p] = original feature index, or -1 for padding."""
    perm = np.full(768, -1, np.int64)
    for kt, (nm, bj) in enumerate(KTS):
        l, packs, n = _unit_info(nm)
        R = MAXR[l]
        dst = 0
        for bi, off in packs:
            f0 = _orig_block_off(l, bi, bj)
            perm[kt * 128 + dst:kt * 128 + dst + R] = np.arange(f0, f0 + R)
            dst += R
    return perm


PERM = _padded_map()


# ---------------------------------------------------------------- host consts
def _unit_A(name):
    """Parity-split 0/1 interval operator: (111, 2, n_rows) fp16.
    at[p, par, off+r] = 1 iff (2p+par) is in [bi*F + r, (bi+1)*F - r)."""
    l, packs, n = _unit_info(name)
    F, R = FBLK[l], MAXR[l]
    A = np.zeros((n, IMAGE), np.float32)
    for bi, off in packs:
        for r in range(R):
            A[off + r, bi * F + r:(bi + 1) * F - r] = 1.0
    A = A.T.reshape(HP, 2, n)
    return np.ascontiguousarray(A.astype(np.float16))


def _unit_M(name):
    """Mask (n_rows, 2 imgs, l, F) fp16, zero on pad rows."""
    l, packs, n = _unit_info(name)
    F, R = FBLK[l], MAXR[l]
    M = np.zeros((n, F), np.float32)
    for _, off in packs:
        for r in range(R):
            M[off + r, r:F - r] = 1.0
    out = np.broadcast_to(M[:, None, None, :], (n, 2, l, F))
    return np.ascontiguousarray(out.astype(np.float16))


def _build_UTS(w1, w2, w3):
    """Stacked phase-2 operators (128, 6, 128) fp32.
    uts[r, kt, d] so that c_kt[f, b] = sum_r uts[r, kt, f] * q_unit[r, b]."""
    ws = {1: np.asarray(w1, np.float32).reshape(1, -1),
          2: np.asarray(w2, np.float32),
          3: np.asarray(w3, np.float32)}
    out = np.zeros((128, 6, 128), np.float32)
    for kt, (nm, bj) in enumerate(KTS):
        l, packs, n = _unit_info(nm)
        F, R = FBLK[l], MAXR[l]
        dst = 0
        for bi, off in packs:
            blk = bi * l + bj
            wv = np.array([ws[l][blk][(F - 1) // 2 - d] for d in range(R)],
                          np.float32)
            U = np.zeros((R, R), np.float32)
            for r in range(R):
                U[r, r] = wv[r]
                U[r, r + 1:] = wv[r + 1:] - wv[r:-1]
            out[off:off + R, kt, dst:dst + R] = U.T
            dst += R
    return out


def _pack_weight(w, b, kin_pad, perm=None):
    """(nout, kin) torch-layout weight + bias -> (128, kt, nout) fp16 with
    k rows permuted by `perm` (padded feature -> original), bias NOT
    included (applied via ACT)."""
    w = np.asarray(w, np.float32)
    nout, kin = w.shape
    kts = kin_pad // 128
    out = np.zeros((128, kts, nout), np.float32)
    for kp in range(kin_pad):
        src = perm[kp] if perm is not None else (kp if kp < kin else -1)
        if src >= 0:
            out[kp % 128, kp // 128, :] = w[:, src]
    return np.ascontiguousarray(out.astype(np.float16))


def _pack_w5(w, b):
    """fc5 (1000, 128) + bias -> (128, 2, 1000) fp16; kt1 row0 = bias."""
    w = np.asarray(w, np.float32)
    out = np.zeros((128, 2, NUM_CLASSES), np.float32)
    out[:, 0, :] = w.T
    out[0, 1, :] = np.asarray(b, np.float32)
    return np.ascontiguousarray(out.astype(np.float16))


def _pack_biases(fcs):
    """fc1-fc4 biases -> (128, 29) fp32 columns [fc1 x16, fc2 x8, fc3 x4, fc4]."""
    out = np.zeros((128, 29), np.float32)
    col = 0
    for i, (w, b) in enumerate(fcs[:4]):
        b = np.asarray(b, np.float32)
        for ot in range((FC_DIMS[i + 1] + 127) // 128):
            n0 = ot * 128
            cnt = min(128, FC_DIMS[i + 1] - n0)
            out[0:cnt, col] = b[n0:n0 + cnt]
            col += 1
    assert col == 29
    return out


def _pack_gmbe(gm, be):
    """gamma/beta -> (128, 12) fp32 in padded-permuted order; pads get
    gamma=0 beta=0 (their c is 0 and stays 0)."""
    gm = np.asarray(gm, np.float32)
    be = np.asarray(be, np.float32)
    out = np.zeros((128, 12), np.float32)
    for kp in range(768):
        f = PERM[kp]
        if f >= 0:
            out[kp % 128, kp // 128] = gm[f]
            out[kp % 128, 6 + kp // 128] = be[f]
    return out


# ---------------------------------------------------------------- bass build
def build_program(n_cores=N_CORES, img_per_core=IMG_PER_CORE):
    B = img_per_core
    NG = B // 2
    nc = bacc.Bacc("TRN2", target_bir_lowering=False, debug=False,
                   num_devices=n_cores)

    # ---- DRAM I/O
    x = nc.dram_tensor("x", [HP, B, 1332], F16, kind="ExternalInput")
    at_d = {nm: nc.dram_tensor(f"at_{nm}", [HP, 2, n], F16,
                               kind="ExternalInput")
            for nm, l, pk, n in UNITS}
    mk_d = {nm: nc.dram_tensor(f"mk_{nm}", [n, 2, l, FBLK[l]], F16,
                               kind="ExternalInput")
            for nm, l, pk, n in UNITS}
    uts_d = nc.dram_tensor("uts", [128, 6, 128], F32, kind="ExternalInput")
    wt_d = {}
    for i in range(1, 5):
        kts = FC_DIMS[i - 1] // 128 + (0 if FC_DIMS[i - 1] % 128 == 0 else 1)
        wt_d[i] = nc.dram_tensor(f"w{i}p", [128, kts, FC_DIMS[i]], F16,
                                 kind="ExternalInput")
    wt_d[5] = nc.dram_tensor("w5p", [128, 2, NUM_CLASSES], F16,
                             kind="ExternalInput")
    bi_d = nc.dram_tensor("biasp", [128, 29], F32, kind="ExternalInput")
    gb_d = nc.dram_tensor("gmbe", [128, 12], F32, kind="ExternalInput")
    on_d = nc.dram_tensor("ones", [1, B], F16, kind="ExternalInput")
    out_d = nc.dram_tensor("out", [B, NUM_CLASSES], F32, kind="ExternalOutput")

    AX = mybir.AxisListType
    OP = mybir.AluOpType
    ACT = mybir.ActivationFunctionType

    UINFO = [(nm, l, n) for nm, l, pk, n in UNITS]

    with tile.TileContext(nc) as tc:
        with tc.tile_pool(name="consts", bufs=1) as cp_pool, \
             tc.tile_pool(name="scr", bufs=3) as scr_pool, \
             tc.tile_pool(name="dram", bufs=1, space="DRAM") as dram_pool:

            # ---- weights + late consts prefetch on the vector queue
            w_sb = {}
            for i in range(1, 6):
                kts = wt_d[i].shape[1]
                w_sb[i] = cp_pool.tile([128, kts, FC_DIMS[i]], F16,
                                       name=f"w{i}_sb")
                nc.scalar.dma_start(out=w_sb[i][:], in_=wt_d[i].ap())
            uts_sb = cp_pool.tile([128, 6, 128], F32, name="uts_sb")
            nc.scalar.dma_start(out=uts_sb[:], in_=uts_d.ap())
            bi_sb = cp_pool.tile([128, 29], F32, name="bi_sb")
            nc.scalar.dma_start(out=bi_sb[:], in_=bi_d.ap())
            gb_sb = cp_pool.tile([128, 12], F32, name="gb_sb")
            nc.scalar.dma_start(out=gb_sb[:], in_=gb_d.ap())


            # ---- phase-1 consts on the sync queue (needed immediately)
            at_sb, mk_sb, q_sb = {}, {}, {}
            for nm, l, n in UINFO:
                at_sb[nm] = cp_pool.tile([HP, 2, n], F16, name=f"at_{nm}_sb")
                nc.sync.dma_start(out=at_sb[nm][:], in_=at_d[nm].ap())
                mk_sb[nm] = cp_pool.tile([n, 2, l, FBLK[l]], F16,
                                         name=f"mk_{nm}_sb")
                nc.sync.dma_start(out=mk_sb[nm][:], in_=mk_d[nm].ap())
                q_sb[nm] = cp_pool.tile([n, B, l], F32, name=f"q_{nm}")

            # ---- whole x shard SBUF-resident: 8 chunked DMAs with 10.6 KB
            # descriptors (big descriptors fan out across all DMA engines;
            # 2.6 KB ones get pinned to a 3-engine subset at ~66 GB/s)
            xg = cp_pool.tile([HP, B, 1332], F16, name="xg_all")
            for ci in range(8):
                i0 = ci * 4
                eng = (nc.sync, nc.scalar)[ci % 2] if ci < 6 else nc.gpsimd
                eng.dma_start(out=xg[0:HP, i0:i0 + 4, :],
                              in_=x.ap()[0:HP, i0:i0 + 4, :])

            # ---- phase 1: segment-reduce to Q from the resident x tile
            with tc.tile_pool(name="zp", bufs=2, space="PSUM") as zp_pool:
                for g in range(NG):
                    for nm, l, n in UINFO:
                        F = FBLK[l]
                        ch = l - 1
                        zp = zp_pool.tile([n, 2, l, F], F32, tag=f"z{nm}")
                        for par in (0, 1):
                            # rhs streams (img, col) = (img, bj, jj) in order
                            nc.tensor.matmul(
                                zp[:], at_sb[nm][0:HP, par, :],
                                xg[0:HP, 2 * g:2 * g + 2,
                                   par * 666 + ch:par * 666 + ch + 664:3],
                                start=(par == 0), stop=(par == 1))
                        sc = scr_pool.tile([n, 2, l, F], F16, tag=f"sc{nm}")
                        nc.scalar.copy(sc[:], zp[:])
                        mm = scr_pool.tile([n, 2, l, F], F16, tag=f"mm{nm}")
                        me = nc.gpsimd if nm == "u2" else nc.vector
                        me.tensor_tensor(mm[:], sc[:], mk_sb[nm][:], OP.mult)
                        nc.vector.tensor_reduce(
                            q_sb[nm][0:n, 2 * g:2 * g + 2, 0:l],
                            mm[:], AX.X, OP.add)

            # ---- phase 2: c_kt = UTS_kt^T @ q  (padded 768-feature space)
            c_ps = []
            stats = cp_pool.tile([128, 12], F32, name="stats")
            sqscr = cp_pool.tile([128, B], F32, name="sqscr")
            c_sb = cp_pool.tile([128, 6, B], F32, name="c_sb")
            with tc.tile_pool(name="cps", bufs=1, space="PSUM") as cps_pool:
                for kt, (nm, bj) in enumerate(KTS):
                    l, _, n = _unit_info(nm)
                    cps = cps_pool.tile([128, B], F32, tag=f"c{kt}")
                    nc.tensor.matmul(
                        cps[:], uts_sb[0:n, kt, :], q_sb[nm][0:n, :, bj],
                        start=True, stop=True)
                    c_ps.append(cps)
                    # ---- phase 3: batch-norm partial stats per k-tile
                    nc.scalar.copy(c_sb[0:128, kt, :], cps[:])
                    nc.vector.tensor_reduce(
                        stats[0:128, kt:kt + 1], cps[:], AX.X, OP.add)
                    nc.vector.tensor_tensor(
                        sqscr[:], c_sb[0:128, kt, :], c_sb[0:128, kt, :],
                        OP.mult)
                    nc.vector.tensor_reduce(
                        stats[0:128, 6 + kt:7 + kt], sqscr[:], AX.X, OP.add)

                # ---- AllReduce of stats
                cc_in = dram_pool.tile([128, 12], F32)
                cc_out = dram_pool.tile(
                    [128, 12], F32,
                    addr_space="Shared" if n_cores > 4 else "Local")
                nc.scalar.dma_start(out=cc_in[:], in_=stats[:])
                nc.gpsimd.collective_compute(
                    "AllReduce", OP.add,
                    replica_groups=[list(range(n_cores))],
                    ins=[cc_in[:].opt()], outs=[cc_out[:].opt()])
                statg = cp_pool.tile([128, 12], F32, name="statg")
                nc.scalar.dma_start(out=statg[:], in_=cc_out[:])

            # ---- phase 4: d = gamma * rsqrt(var + eps); cn = d*c + e
            nb = float(n_cores * B)
            bnd = cp_pool.tile([128, 6], F32, name="bnd")
            bne = cp_pool.tile([128, 6], F32, name="bne")
            mu = cp_pool.tile([128, 6], F32, name="mu")
            vtmp = cp_pool.tile([128, 6], F32, name="vtmp")
            nc.scalar.mul(mu[:], statg[0:128, 0:6], 1.0 / nb)
            nc.scalar.mul(vtmp[:], statg[0:128, 6:12], 1.0 / nb)
            nc.vector.tensor_tensor(bnd[:], mu[:], mu[:], OP.mult)
            nc.vector.tensor_tensor(vtmp[:], vtmp[:], bnd[:], OP.subtract)
            eps = cp_pool.tile([128, 1], F32, name="eps")
            nc.vector.memset(eps[:], 1e-5)
            nc.scalar.activation(vtmp[:], vtmp[:], ACT.Sqrt, bias=eps[:])
            nc.vector.reciprocal(vtmp[:], vtmp[:])
            nc.vector.tensor_tensor(bnd[:], gb_sb[0:128, 0:6], vtmp[:], OP.mult)
            nc.vector.tensor_tensor(vtmp[:], mu[:], bnd[:], OP.mult)
            nc.vector.tensor_tensor(bne[:], gb_sb[0:128, 6:12], vtmp[:],
                                    OP.subtract)
            cn_sb = cp_pool.tile([128, 6, B], F16, name="cn_sb")
            for kt in range(6):
                nc.vector.tensor_scalar(
                    out=cn_sb[0:128, kt, :], in0=c_sb[0:128, kt, :],
                    scalar1=bnd[0:128, kt:kt + 1],
                    scalar2=bne[0:128, kt:kt + 1],
                    op0=OP.mult, op1=OP.add)

            # ---- phase 5: feature-major MLP, stationary SBUF weights
            h = cn_sb
            bcol = 0
            with tc.tile_pool(name="mps", bufs=4, space="PSUM") as mps_pool:
                for i in range(1, 5):
                    kts = w_sb[i].shape[1]
                    nout = FC_DIMS[i]
                    nots = (nout + 127) // 128
                    extra = 1 if i == 4 else 0   # fc4 output carries ones kt
                    hn = cp_pool.tile([128, nots + extra, B], F16,
                                      name=f"h{i}")
                    for ot in range(nots):
                        n0 = ot * 128
                        cnt = min(128, nout - n0)
                        ps = mps_pool.tile([cnt, B], F32, tag="mp",
                                           name=f"mp{i}_{ot}")
                        for kt in range(kts):
                            nc.tensor.matmul(
                                ps[:], w_sb[i][0:128, kt, n0:n0 + cnt],
                                h[0:128, kt, :],
                                start=(kt == 0), stop=(kt == kts - 1))
                        nc.scalar.activation(
                            hn[0:cnt, ot, :], ps[:], ACT.Relu,
                            bias=bi_sb[0:cnt, bcol + ot:bcol + ot + 1])
                    bcol += nots
                    h = hn
                # fc4 ones k-tile for the fc5 bias row
                nc.sync.dma_start(out=h[0:1, 1, :], in_=on_d.ap())

                # fc5 batch-major: h4 k-tiles stationary, W5 moving
                out_sb = cp_pool.tile([B, NUM_CLASSES], F32, name="out_sb")
                for half in range(2):
                    n0, n1 = half * 500, 500 + half * 500
                    ps = mps_pool.tile([B, 500], F32, tag="mp",
                                       name=f"mp5_{half}")
                    for kt, kp in ((0, 128), (1, 1)):
                        nc.tensor.matmul(
                            ps[:], h[0:kp, kt, :], w_sb[5][0:kp, kt, n0:n1],
                            start=(kt == 0), stop=(kt == 1))
                    nc.scalar.copy(out_sb[0:B, n0:n1], ps[:])
            nc.sync.dma_start(out=out_d.ap(), in_=out_sb[:])

    nc.compile()
    return nc


# ------------------------------------------------------------------- runtime
_CACHE = {}


def _get_program():
    key = (N_CORES, IMG_PER_CORE)
    if key not in _CACHE:
        _CACHE[key] = build_program(*key)
    return _CACHE[key]


def _host_consts(w1, w2, w3, bn_gamma, bn_beta, fcs):
    consts = {}
    for nm, l, pk, n in UNITS:
        consts[f"at_{nm}"] = _unit_A(nm)
        consts[f"mk_{nm}"] = _unit_M(nm)
    consts["uts"] = _build_UTS(w1, w2, w3)
    consts["w1p"] = _pack_weight(fcs[0][0], fcs[0][1], 768, perm=PERM)
    consts["w2p"] = _pack_weight(fcs[1][0], fcs[1][1], 2048)
    consts["w3p"] = _pack_weight(fcs[2][0], fcs[2][1], 1024)
    consts["w4p"] = _pack_weight(fcs[3][0], fcs[3][1], 512)
    consts["w5p"] = _pack_w5(fcs[4][0], fcs[4][1])
    consts["biasp"] = _pack_biases(fcs)
    consts["gmbe"] = _pack_gmbe(bn_gamma, bn_beta)
    consts["ones"] = np.ones((1, IMG_PER_CORE), np.float16)
    return consts


def kernel(x, w1, w2, w3, bn_gamma, bn_beta,
           fc1_w, fc1_b, fc2_w, fc2_b, fc3_w, fc3_b, fc4_w, fc4_b,
           fc5_w, fc5_b):
    from concourse.bass_utils import run_bass_kernel_spmd

    nc = _get_program()
    consts = _host_consts(
        w1, w2, w3, bn_gamma, bn_beta,
        [(fc1_w, fc1_b), (fc2_w, fc2_b), (fc3_w, fc3_b), (fc4_w, fc4_b),
         (fc5_w, fc5_b)])
    xh = np.asarray(x, np.float32).astype(np.float16)
    xh = xh.reshape(N_CORES, IMG_PER_CORE, HP, 1332)
    in_maps = []
    for s in range(N_CORES):
        m = dict(consts)
        m["x"] = np.ascontiguousarray(xh[s].transpose(1, 0, 2))
        in_maps.append(m)

    trace = bool(int(os.environ.get("BASSDFT_TRACE", "0")))
    if trace:
        _install_ntff_hook()
    res = run_bass_kernel_spmd(nc, in_maps, core_ids=list(range(N_CORES)),
                               trace=trace)
    if trace:
        kernel.last_exec_time_ns = res.exec_time_ns
        kernel.last_results = res
    return np.concatenate([res.results[s]["out"] for s in range(N_CORES)],
                          axis=0)


def _install_ntff_hook():
    """Register the axon NTFF profiling hook (antenv.axon_hooks is absent in
    this image) and disable the share-bucket artifact upload."""
    try:
        from antenv import axon_hooks  # noqa: F401
        return
    except ImportError:
        pass
    try:
        from trn_agent_boot.trn_boot import _ntff_profile_via_ctypes
    except ImportError:
        return
    import antenv
    import concourse.bass_utils as bu
    mod = types.ModuleType("antenv.axon_hooks")
    hook = [_ntff_profile_via_ctypes("/opt/axon/libaxon_pjrt.so")]
    mod.get_axon_ntff_profile_hook = lambda: hook[0]
    mod.set_axon_ntff_profile_hook = lambda h: hook.__setitem__(0, h)
    sys.modules["antenv.axon_hooks"] = mod
    antenv.axon_hooks = mod
    bu.upload_artifacts = lambda tmpdir: tmpdir


# revision 18
# speedup vs baseline: 1.0297x; 1.0297x over previous
"""Trainium2 Bass kernel for nn_DFTModel (segment_reduce), v2.

Math: for each level l in {1,2,3} the reference multiplies channel l-1 by a
block-tiled radial ("square ring") filter and sums nested square rectangles.
Nested-square sums of the raw image Q[r] are computed as
    Q = rowsum(M o (A~ @ X))       (PE matmul with 0/1 interval operator A~,
                                    mask-multiply + axis reduce)
and the filter weights fold into a tiny per-block upper-triangular transform
    c = U @ Q
so the 222x222 filter product is never materialized.

v2 layout/schedule changes vs v1 (which was serialized on the Sync queue's
DMA issue rate at ~94 GB/s effective):
  * x is converted to float16 on the host and stored 2 rows per partition
    (111 partitions x 1332 elems, one contiguous 2664 B descriptor per
    partition) - half the bytes and half the DMA descriptors. The row-parity
    split becomes two accumulating matmuls per unit with parity-split A~
    operators.
  * image DMAs are spread across two queues (sync early, gpsimd late - the
    gpsimd queue sits behind the startup all-core barrier); MLP weights are
    prefetched in fp16 on the vector queue at t=0 and stay SBUF-resident.
  * level 3 is repacked into ONE 111-row unit (row offsets 0/37/74). All
    engine APs start at partition base 0, so no DMA scatter is needed
    anywhere: phase 2 is six stacked-operator matmuls (one per 128-feature
    tile of a padded 768-feature space), and fc1 weights / bn params are
    host-permuted to match.
  * phase-1 masked reduce is split across engines: ACT casts PSUM z to fp16
    in SBUF, mask-multiplies run on DVE (2x fp16 mode) for early groups and
    on GpSimd for late groups, DVE does the axis reduces into fp32 q.
  * the MLP runs feature-major with stationary fp16 weights (out[n, img]),
    biases applied by the ACT engine per partition - no transposes, no ones
    rows except fc5, whose bias rides as an extra weight k-tile against an
    all-ones activation row.
BatchNorm batch statistics are exchanged with one 6 KB AllReduce.
"""

import os
import sys
import types

import numpy as np

for _p in ("/opt/trn_rl_repo",):
    if _p not in sys.path and os.path.isdir(_p):
        sys.path.insert(0, _p)

import concourse.bacc as bacc
import concourse.bass as bass
import concourse.mybir as mybir
import concourse.tile as tile

F32 = mybir.dt.float32
F16 = mybir.dt.float16

IMAGE = 222
HP = 111                      # partitions for the 2-rows-per-partition layout
N_CORES = 8
IMG_PER_CORE = 32
NUM_CLASSES = 1000
FC_DIMS = [768, 2048, 1024, 512, 128, 1000]   # 768 = padded feature count


def _get_dim(level):
    d = IMAGE / level
    if d % 2 == 1:
        d += 1
    return int(d // 2)


MAXR = {l: _get_dim(l) for l in (1, 2, 3)}          # 111, 56, 37
FBLK = {l: IMAGE // l for l in (1, 2, 3)}           # 222, 111, 74
NUM_COEFF = sum(l * l * MAXR[l] for l in (1, 2, 3))  # 668

# units: (name, level, [(bi, row_offset)...], n_rows)
UNITS = [
    ("u1", 1, [(0, 0)], 111),
    ("u2", 2, [(0, 0), (1, 64)], 120),
    ("u3", 3, [(0, 0), (1, 37), (2, 74)], 111),
]

# phase-2 k-tiles of the padded 768-feature space: (q-unit, bj)
KTS = [("u1", 0), ("u2", 0), ("u2", 1), ("u3", 0), ("u3", 1), ("u3", 2)]


def _unit_info(name):
    for nm, lv, pk, nr in UNITS:
        if nm == name:
            return lv, pk, nr
    raise KeyError(name)


def _orig_block_off(l, bi, bj):
    base = {1: 0, 2: 111, 3: 335}[l]
    return base + (bi * l + bj) * MAXR[l]


def _padded_map():
    """perm[kt*128 + p] = original feature index, or -1 for padding."""
    perm = np.full(768, -1, np.int64)
    for kt, (nm, bj) in enumerate(KTS):
        l, packs, n = _unit_info(nm)
        R = MAXR[l]
        dst = 0
        for bi, off in packs:
            f0 = _orig_block_off(l, bi, bj)
            perm[kt * 128 + dst:kt * 128 + dst + R] = np.arange(f0, f0 + R)
            dst += R
    return perm


PERM = _padded_map()


# ---------------------------------------------------------------- host consts
def _unit_A(name):
    """Parity-split 0/1 interval operator: (111, 2, n_rows) fp16.
    at[p, par, off+r] = 1 iff (2p+par) is in [bi*F + r, (bi+1)*F - r)."""
    l, packs, n = _unit_info(name)
    F, R = FBLK[l], MAXR[l]
    A = np.zeros((n, IMAGE), np.float32)
    for bi, off in packs:
        for r in range(R):
            A[off + r, bi * F + r:(bi + 1) * F - r] = 1.0
    A = A.T.reshape(HP, 2, n)
    return np.ascontiguousarray(A.astype(np.float16))


def _unit_M(name):
    """Mask (n_rows, 2 imgs, l, F) fp16, zero on pad rows."""
    l, packs, n = _unit_info(name)
    F, R = FBLK[l], MAXR[l]
    M = np.zeros((n, F), np.float32)
    for _, off in packs:
        for r in range(R):
            M[off + r, r:F - r] = 1.0
    out = np.broadcast_to(M[:, None, None, :], (n, 2, l, F))
    return np.ascontiguousarray(out.astype(np.float16))


def _build_UTS(w1, w2, w3):
    """Stacked phase-2 operators (128, 6, 128) fp32.
    uts[r, kt, d] so that c_kt[f, b] = sum_r uts[r, kt, f] * q_unit[r, b]."""
    ws = {1: np.asarray(w1, np.float32).reshape(1, -1),
          2: np.asarray(w2, np.float32),
          3: np.asarray(w3, np.float32)}
    out = np.zeros((128, 6, 128), np.float32)
    for kt, (nm, bj) in enumerate(KTS):
        l, packs, n = _unit_info(nm)
        F, R = FBLK[l], MAXR[l]
        dst = 0
        for bi, off in packs:
            blk = bi * l + bj
            wv = np.array([ws[l][blk][(F - 1) // 2 - d] for d in range(R)],
                          np.float32)
            U = np.zeros((R, R), np.float32)
            for r in range(R):
                U[r, r] = wv[r]
                U[r, r + 1:] = wv[r + 1:] - wv[r:-1]
            out[off:off + R, kt, dst:dst + R] = U.T
            dst += R
    return out


def _pack_weight(w, b, kin_pad, perm=None):
    """(nout, kin) torch-layout weight + bias -> (128, kt, nout) fp16 with
    k rows permuted by `perm` (padded feature -> original), bias NOT
    included (applied via ACT)."""
    w = np.asarray(w, np.float32)
    nout, kin = w.shape
    kts = kin_pad // 128
    out = np.zeros((128, kts, nout), np.float32)
    for kp in range(kin_pad):
        src = perm[kp] if perm is not None else (kp if kp < kin else -1)
        if src >= 0:
            out[kp % 128, kp // 128, :] = w[:, src]
    return np.ascontiguousarray(out.astype(np.float16))


def _pack_w5(w, b):
    """fc5 (1000, 128) + bias -> (128, 2, 1000) fp16; kt1 row0 = bias."""
    w = np.asarray(w, np.float32)
    out = np.zeros((128, 2, NUM_CLASSES), np.float32)
    out[:, 0, :] = w.T
    out[0, 1, :] = np.asarray(b, np.float32)
    return np.ascontiguousarray(out.astype(np.float16))


def _pack_biases(fcs):
    """fc1-fc4 biases -> (128, 29) fp32 columns [fc1 x16, fc2 x8, fc3 x4, fc4]."""
    out = np.zeros((128, 29), np.float32)
    col = 0
    for i, (w, b) in enumerate(fcs[:4]):
        b = np.asarray(b, np.float32)
        for ot in range((FC_DIMS[i + 1] + 127) // 128):
            n0 = ot * 128
            cnt = min(128, FC_DIMS[i + 1] - n0)
            out[0:cnt, col] = b[n0:n0 + cnt]
            col += 1
    assert col == 29
    return out


def _pack_gmbe(gm, be):
    """gamma/beta -> (128, 12) fp32 in padded-permuted order; pads get
    gamma=0 beta=0 (their c is 0 and stays 0)."""
    gm = np.asarray(gm, np.float32)
    be = np.asarray(be, np.float32)
    out = np.zeros((128, 12), np.float32)
    for kp in range(768):
        f = PERM[kp]
        if f >= 0:
            out[kp % 128, kp // 128] = gm[f]
            out[kp % 128, 6 + kp // 128] = be[f]
    return out


# ---------------------------------------------------------------- bass build
def build_program(n_cores=N_CORES, img_per_core=IMG_PER_CORE):
    B = img_per_core
    NG = B // 2
    nc = bacc.Bacc("TRN2", target_bir_lowering=False, debug=False,
                   num_devices=n_cores)

    # ---- DRAM I/O
    x_d = [nc.dram_tensor(f"x{k}", [HP, B // 4, 1332], F16,
                          kind="ExternalInput") for k in range(4)]
    at_d = {nm: nc.dram_tensor(f"at_{nm}", [HP, 2, n], F16,
                               kind="ExternalInput")
            for nm, l, pk, n in UNITS}
    mk_d = {nm: nc.dram_tensor(f"mk_{nm}", [n, 2, l, FBLK[l]], F16,
                               kind="ExternalInput")
            for nm, l, pk, n in UNITS}
    uts_d = nc.dram_tensor("uts", [128, 6, 128], F32, kind="ExternalInput")
    wt_d = {}
    for i in range(1, 5):
        kts = FC_DIMS[i - 1] // 128 + (0 if FC_DIMS[i - 1] % 128 == 0 else 1)
        wt_d[i] = nc.dram_tensor(f"w{i}p", [128, kts, FC_DIMS[i]], F16,
                                 kind="ExternalInput")
    wt_d[5] = nc.dram_tensor("w5p", [128, 2, NUM_CLASSES], F16,
                             kind="ExternalInput")
    bi_d = nc.dram_tensor("biasp", [128, 29], F32, kind="ExternalInput")
    gb_d = nc.dram_tensor("gmbe", [128, 12], F32, kind="ExternalInput")
    on_d = nc.dram_tensor("ones", [1, B], F16, kind="ExternalInput")
    out_d = nc.dram_tensor("out", [B, NUM_CLASSES], F32, kind="ExternalOutput")

    AX = mybir.AxisListType
    OP = mybir.AluOpType
    ACT = mybir.ActivationFunctionType

    UINFO = [(nm, l, n) for nm, l, pk, n in UNITS]

    with tile.TileContext(nc) as tc:
        with tc.tile_pool(name="consts", bufs=1) as cp_pool, \
             tc.tile_pool(name="scr", bufs=3) as scr_pool, \
             tc.tile_pool(name="dram", bufs=1, space="DRAM") as dram_pool:

            # ---- weights + late consts prefetch on the vector queue
            w_sb = {}
            for i in range(1, 6):
                kts = wt_d[i].shape[1]
                w_sb[i] = cp_pool.tile([128, kts, FC_DIMS[i]], F16,
                                       name=f"w{i}_sb")
                nc.scalar.dma_start(out=w_sb[i][:], in_=wt_d[i].ap())
            uts_sb = cp_pool.tile([128, 6, 128], F32, name="uts_sb")
            nc.scalar.dma_start(out=uts_sb[:], in_=uts_d.ap())
            bi_sb = cp_pool.tile([128, 29], F32, name="bi_sb")
            nc.scalar.dma_start(out=bi_sb[:], in_=bi_d.ap())
            gb_sb = cp_pool.tile([128, 12], F32, name="gb_sb")
            nc.scalar.dma_start(out=gb_sb[:], in_=gb_d.ap())


            # ---- phase-1 consts on the sync queue (needed immediately)
            at_sb, mk_sb, q_sb = {}, {}, {}
            for nm, l, n in UINFO:
                at_sb[nm] = cp_pool.tile([HP, 2, n], F16, name=f"at_{nm}_sb")
                nc.sync.dma_start(out=at_sb[nm][:], in_=at_d[nm].ap())
                mk_sb[nm] = cp_pool.tile([n, 2, l, FBLK[l]], F16,
                                         name=f"mk_{nm}_sb")
                nc.sync.dma_start(out=mk_sb[nm][:], in_=mk_d[nm].ap())
                q_sb[nm] = cp_pool.tile([n, B, l], F32, name=f"q_{nm}")

            # ---- whole x shard SBUF-resident, loaded from four fully
            # CONTIGUOUS dram blobs: contiguous-walk DMAs round-robin their
            # descriptors across all 16 DMA engines (~350 GB/s), while
            # strided-source DMAs collapse onto a 3-engine subset (~66 GB/s)
            xg = cp_pool.tile([HP, B, 1332], F16, name="xg_all")
            for k in range(4):
                i0 = k * (B // 4)
                eng = (nc.sync, nc.scalar)[k % 2]
                eng.dma_start(out=xg[0:HP, i0:i0 + B // 4, :],
                              in_=x_d[k].ap())

            # ---- phase 1: segment-reduce to Q from the resident x tile
            with tc.tile_pool(name="zp", bufs=2, space="PSUM") as zp_pool:
                for g in range(NG):
                    for nm, l, n in UINFO:
                        F = FBLK[l]
                        ch = l - 1
                        zp = zp_pool.tile([n, 2, l, F], F32, tag=f"z{nm}")
                        for par in (0, 1):
                            # rhs streams (img, col) = (img, bj, jj) in order
                            nc.tensor.matmul(
                                zp[:], at_sb[nm][0:HP, par, :],
                                xg[0:HP, 2 * g:2 * g + 2,
                                   par * 666 + ch:par * 666 + ch + 664:3],
                                start=(par == 0), stop=(par == 1))
                        sc = scr_pool.tile([n, 2, l, F], F16, tag=f"sc{nm}")
                        nc.scalar.copy(sc[:], zp[:])
                        mm = scr_pool.tile([n, 2, l, F], F16, tag=f"mm{nm}")
                        me = (nc.gpsimd if (nm == "u2" and g >= 8)
                              else nc.vector)
                        me.tensor_tensor(mm[:], sc[:], mk_sb[nm][:], OP.mult)
                        nc.vector.tensor_reduce(
                            q_sb[nm][0:n, 2 * g:2 * g + 2, 0:l],
                            mm[:], AX.X, OP.add)

            # ---- phase 2: c_kt = UTS_kt^T @ q  (padded 768-feature space)
            c_ps = []
            stats = cp_pool.tile([128, 12], F32, name="stats")
            sqscr = cp_pool.tile([128, B], F32, name="sqscr")
            c_sb = cp_pool.tile([128, 6, B], F32, name="c_sb")
            with tc.tile_pool(name="cps", bufs=1, space="PSUM") as cps_pool:
                for kt, (nm, bj) in enumerate(KTS):
                    l, _, n = _unit_info(nm)
                    cps = cps_pool.tile([128, B], F32, tag=f"c{kt}")
                    nc.tensor.matmul(
                        cps[:], uts_sb[0:n, kt, :], q_sb[nm][0:n, :, bj],
                        start=True, stop=True)
                    c_ps.append(cps)
                    # ---- phase 3: batch-norm partial stats per k-tile
                    nc.scalar.copy(c_sb[0:128, kt, :], cps[:])
                    nc.vector.tensor_reduce(
                        stats[0:128, kt:kt + 1], cps[:], AX.X, OP.add)
                    nc.vector.tensor_tensor(
                        sqscr[:], c_sb[0:128, kt, :], c_sb[0:128, kt, :],
                        OP.mult)
                    nc.vector.tensor_reduce(
                        stats[0:128, 6 + kt:7 + kt], sqscr[:], AX.X, OP.add)

                # ---- AllReduce of stats
                cc_in = dram_pool.tile([128, 12], F32)
                cc_out = dram_pool.tile(
                    [128, 12], F32,
                    addr_space="Shared" if n_cores > 4 else "Local")
                nc.scalar.dma_start(out=cc_in[:], in_=stats[:])
                nc.gpsimd.collective_compute(
                    "AllReduce", OP.add,
                    replica_groups=[list(range(n_cores))],
                    ins=[cc_in[:].opt()], outs=[cc_out[:].opt()])
                statg = cp_pool.tile([128, 12], F32, name="statg")
                nc.scalar.dma_start(out=statg[:], in_=cc_out[:])

            # ---- phase 4: d = gamma * rsqrt(var + eps); cn = d*c + e
            nb = float(n_cores * B)
            bnd = cp_pool.tile([128, 6], F32, name="bnd")
            bne = cp_pool.tile([128, 6], F32, name="bne")
            mu = cp_pool.tile([128, 6], F32, name="mu")
            vtmp = cp_pool.tile([128, 6], F32, name="vtmp")
            nc.scalar.mul(mu[:], statg[0:128, 0:6], 1.0 / nb)
            nc.scalar.mul(vtmp[:], statg[0:128, 6:12], 1.0 / nb)
            nc.vector.tensor_tensor(bnd[:], mu[:], mu[:], OP.mult)
            nc.vector.tensor_tensor(vtmp[:], vtmp[:], bnd[:], OP.subtract)
            eps = cp_pool.tile([128, 1], F32, name="eps")
            nc.vector.memset(eps[:], 1e-5)
            nc.scalar.activation(vtmp[:], vtmp[:], ACT.Sqrt, bias=eps[:])
            nc.vector.reciprocal(vtmp[:], vtmp[:])
            nc.vector.tensor_tensor(bnd[:], gb_sb[0:128, 0:6], vtmp[:], OP.mult)
            nc.vector.tensor_tensor(vtmp[:], mu[:], bnd[:], OP.mult)
            nc.vector.tensor_tensor(bne[:], gb_sb[0:128, 6:12], vtmp[:],
                                    OP.subtract)
            cn_sb = cp_pool.tile([128, 6, B], F16, name="cn_sb")
            for kt in range(6):
                nc.vector.tensor_scalar(
                    out=cn_sb[0:128, kt, :], in0=c_sb[0:128, kt, :],
                    scalar1=bnd[0:128, kt:kt + 1],
                    scalar2=bne[0:128, kt:kt + 1],
                    op0=OP.mult, op1=OP.add)

            # ---- phase 5: feature-major MLP, stationary SBUF weights
            h = cn_sb
            bcol = 0
            with tc.tile_pool(name="mps", bufs=4, space="PSUM") as mps_pool:
                for i in range(1, 5):
                    kts = w_sb[i].shape[1]
                    nout = FC_DIMS[i]
                    nots = (nout + 127) // 128
                    extra = 1 if i == 4 else 0   # fc4 output carries ones kt
                    hn = cp_pool.tile([128, nots + extra, B], F16,
                                      name=f"h{i}")
                    for ot in range(nots):
                        n0 = ot * 128
                        cnt = min(128, nout - n0)
                        ps = mps_pool.tile([cnt, B], F32, tag="mp",
                                           name=f"mp{i}_{ot}")
                        for kt in range(kts):
                            nc.tensor.matmul(
                                ps[:], w_sb[i][0:128, kt, n0:n0 + cnt],
                                h[0:128, kt, :],
                                start=(kt == 0), stop=(kt == kts - 1))
                        nc.scalar.activation(
                            hn[0:cnt, ot, :], ps[:], ACT.Relu,
                            bias=bi_sb[0:cnt, bcol + ot:bcol + ot + 1])
                    bcol += nots
                    h = hn
                # fc4 ones k-tile for the fc5 bias row
                nc.sync.dma_start(out=h[0:1, 1, :], in_=on_d.ap())

                # fc5 batch-major: h4 k-tiles stationary, W5 moving
                out_sb = cp_pool.tile([B, NUM_CLASSES], F32, name="out_sb")
                for half in range(2):
                    n0, n1 = half * 500, 500 + half * 500
                    ps = mps_pool.tile([B, 500], F32, tag="mp",
                                       name=f"mp5_{half}")
                    for kt, kp in ((0, 128), (1, 1)):
                        nc.tensor.matmul(
                            ps[:], h[0:kp, kt, :], w_sb[5][0:kp, kt, n0:n1],
                            start=(kt == 0), stop=(kt == 1))
                    nc.scalar.copy(out_sb[0:B, n0:n1], ps[:])
            nc.sync.dma_start(out=out_d.ap(), in_=out_sb[:])

    nc.compile()
    return nc


# ------------------------------------------------------------------- runtime
_CACHE = {}


def _get_program():
    key = (N_CORES, IMG_PER_CORE)
    if key not in _CACHE:
        _CACHE[key] = build_program(*key)
    return _CACHE[key]


def _host_consts(w1, w2, w3, bn_gamma, bn_beta, fcs):
    consts = {}
    for nm, l, pk, n in UNITS:
        consts[f"at_{nm}"] = _unit_A(nm)
        consts[f"mk_{nm}"] = _unit_M(nm)
    consts["uts"] = _build_UTS(w1, w2, w3)
    consts["w1p"] = _pack_weight(fcs[0][0], fcs[0][1], 768, perm=PERM)
    consts["w2p"] = _pack_weight(fcs[1][0], fcs[1][1], 2048)
    consts["w3p"] = _pack_weight(fcs[2][0], fcs[2][1], 1024)
    consts["w4p"] = _pack_weight(fcs[3][0], fcs[3][1], 512)
    consts["w5p"] = _pack_w5(fcs[4][0], fcs[4][1])
    consts["biasp"] = _pack_biases(fcs)
    consts["gmbe"] = _pack_gmbe(bn_gamma, bn_beta)
    consts["ones"] = np.ones((1, IMG_PER_CORE), np.float16)
    return consts


def kernel(x, w1, w2, w3, bn_gamma, bn_beta,
           fc1_w, fc1_b, fc2_w, fc2_b, fc3_w, fc3_b, fc4_w, fc4_b,
           fc5_w, fc5_b):
    from concourse.bass_utils import run_bass_kernel_spmd

    nc = _get_program()
    consts = _host_consts(
        w1, w2, w3, bn_gamma, bn_beta,
        [(fc1_w, fc1_b), (fc2_w, fc2_b), (fc3_w, fc3_b), (fc4_w, fc4_b),
         (fc5_w, fc5_b)])
    xh = np.asarray(x, np.float32).astype(np.float16)
    xh = xh.reshape(N_CORES, IMG_PER_CORE, HP, 1332)
    q4 = IMG_PER_CORE // 4
    in_maps = []
    for s in range(N_CORES):
        m = dict(consts)
        xt = xh[s].transpose(1, 0, 2)
        for k in range(4):
            m[f"x{k}"] = np.ascontiguousarray(xt[:, k * q4:(k + 1) * q4, :])
        in_maps.append(m)

    trace = bool(int(os.environ.get("BASSDFT_TRACE", "0")))
    if trace:
        _install_ntff_hook()
    res = run_bass_kernel_spmd(nc, in_maps, core_ids=list(range(N_CORES)),
                               trace=trace)
    if trace:
        kernel.last_exec_time_ns = res.exec_time_ns
        kernel.last_results = res
    return np.concatenate([res.results[s]["out"] for s in range(N_CORES)],
                          axis=0)


def _install_ntff_hook():
    """Register the axon NTFF profiling hook (antenv.axon_hooks is absent in
    this image) and disable the share-bucket artifact upload."""
    try:
        from antenv import axon_hooks  # noqa: F401
        return
    except ImportError:
        pass
    try:
        from trn_agent_boot.trn_boot import _ntff_profile_via_ctypes
    except ImportError:
        return
    import antenv
    import concourse.bass_utils as bu
    mod = types.ModuleType("antenv.axon_hooks")
    hook = [_ntff_profile_via_ctypes("/opt/axon/libaxon_pjrt.so")]
    mod.get_axon_ntff_profile_hook = lambda: hook[0]
    mod.set_axon_ntff_profile_hook = lambda h: hook.__setitem__(0, h)
    sys.modules["antenv.axon_hooks"] = mod
    antenv.axon_hooks = mod
    bu.upload_artifacts = lambda tmpdir: tmpdir


# revision 19
# speedup vs baseline: 1.4285x; 1.3873x over previous
"""Trainium2 Bass kernel for nn_DFTModel (segment_reduce), v2.

Math: for each level l in {1,2,3} the reference multiplies channel l-1 by a
block-tiled radial ("square ring") filter and sums nested square rectangles.
Nested-square sums of the raw image Q[r] are computed as
    Q = rowsum(M o (A~ @ X))       (PE matmul with 0/1 interval operator A~,
                                    mask-multiply + axis reduce)
and the filter weights fold into a tiny per-block upper-triangular transform
    c = U @ Q
so the 222x222 filter product is never materialized.

v2 layout/schedule changes vs v1 (which was serialized on the Sync queue's
DMA issue rate at ~94 GB/s effective):
  * x is converted to float16 on the host and stored 2 rows per partition
    (111 partitions x 1332 elems, one contiguous 2664 B descriptor per
    partition) - half the bytes and half the DMA descriptors. The row-parity
    split becomes two accumulating matmuls per unit with parity-split A~
    operators.
  * image DMAs are spread across two queues (sync early, gpsimd late - the
    gpsimd queue sits behind the startup all-core barrier); MLP weights are
    prefetched in fp16 on the vector queue at t=0 and stay SBUF-resident.
  * level 3 is repacked into ONE 111-row unit (row offsets 0/37/74). All
    engine APs start at partition base 0, so no DMA scatter is needed
    anywhere: phase 2 is six stacked-operator matmuls (one per 128-feature
    tile of a padded 768-feature space), and fc1 weights / bn params are
    host-permuted to match.
  * phase-1 masked reduce is split across engines: ACT casts PSUM z to fp16
    in SBUF, mask-multiplies run on DVE (2x fp16 mode) for early groups and
    on GpSimd for late groups, DVE does the axis reduces into fp32 q.
  * the MLP runs feature-major with stationary fp16 weights (out[n, img]),
    biases applied by the ACT engine per partition - no transposes, no ones
    rows except fc5, whose bias rides as an extra weight k-tile against an
    all-ones activation row.
BatchNorm batch statistics are exchanged with one 6 KB AllReduce.
"""

import os
import sys
import types

import numpy as np

for _p in ("/opt/trn_rl_repo",):
    if _p not in sys.path and os.path.isdir(_p):
        sys.path.insert(0, _p)

import concourse.bacc as bacc
import concourse.bass as bass
import concourse.mybir as mybir
import concourse.tile as tile

F32 = mybir.dt.float32
F16 = mybir.dt.float16

IMAGE = 222
HP = 111                      # partitions for the 2-rows-per-partition layout
N_CORES = 8
IMG_PER_CORE = 32
NUM_CLASSES = 1000
FC_DIMS = [768, 2048, 1024, 512, 128, 1000]   # 768 = padded feature count


def _get_dim(level):
    d = IMAGE / level
    if d % 2 == 1:
        d += 1
    return int(d // 2)


MAXR = {l: _get_dim(l) for l in (1, 2, 3)}          # 111, 56, 37
FBLK = {l: IMAGE // l for l in (1, 2, 3)}           # 222, 111, 74
NUM_COEFF = sum(l * l * MAXR[l] for l in (1, 2, 3))  # 668

# units: (name, level, [(bi, row_offset)...], n_rows)
UNITS = [
    ("u1", 1, [(0, 0)], 111),
    ("u2", 2, [(0, 0), (1, 64)], 120),
    ("u3", 3, [(0, 0), (1, 37), (2, 74)], 111),
]

# phase-2 k-tiles of the padded 768-feature space: (q-unit, bj)
KTS = [("u1", 0), ("u2", 0), ("u2", 1), ("u3", 0), ("u3", 1), ("u3", 2)]


def _unit_info(name):
    for nm, lv, pk, nr in UNITS:
        if nm == name:
            return lv, pk, nr
    raise KeyError(name)


def _orig_block_off(l, bi, bj):
    base = {1: 0, 2: 111, 3: 335}[l]
    return base + (bi * l + bj) * MAXR[l]


def _padded_map():
    """perm[kt*128 + p] = original feature index, or -1 for padding."""
    perm = np.full(768, -1, np.int64)
    for kt, (nm, bj) in enumerate(KTS):
        l, packs, n = _unit_info(nm)
        R = MAXR[l]
        dst = 0
        for bi, off in packs:
            f0 = _orig_block_off(l, bi, bj)
            perm[kt * 128 + dst:kt * 128 + dst + R] = np.arange(f0, f0 + R)
            dst += R
    return perm


PERM = _padded_map()


# ---------------------------------------------------------------- host consts
def _unit_A(name):
    """Parity-split 0/1 interval operator: (111, 2, n_rows) fp16.
    at[p, par, off+r] = 1 iff (2p+par) is in [bi*F + r, (bi+1)*F - r)."""
    l, packs, n = _unit_info(name)
    F, R = FBLK[l], MAXR[l]
    A = np.zeros((n, IMAGE), np.float32)
    for bi, off in packs:
        for r in range(R):
            A[off + r, bi * F + r:(bi + 1) * F - r] = 1.0
    A = A.T.reshape(HP, 2, n)
    return np.ascontiguousarray(A.astype(np.float16))


def _unit_M(name):
    """Mask (n_rows, 2 imgs, l, F) fp16, zero on pad rows."""
    l, packs, n = _unit_info(name)
    F, R = FBLK[l], MAXR[l]
    M = np.zeros((n, F), np.float32)
    for _, off in packs:
        for r in range(R):
            M[off + r, r:F - r] = 1.0
    out = np.broadcast_to(M[:, None, None, :], (n, 2, l, F))
    return np.ascontiguousarray(out.astype(np.float16))


def _build_UTS(w1, w2, w3):
    """Stacked phase-2 operators (128, 6, 128) fp32.
    uts[r, kt, d] so that c_kt[f, b] = sum_r uts[r, kt, f] * q_unit[r, b]."""
    ws = {1: np.asarray(w1, np.float32).reshape(1, -1),
          2: np.asarray(w2, np.float32),
          3: np.asarray(w3, np.float32)}
    out = np.zeros((128, 6, 128), np.float32)
    for kt, (nm, bj) in enumerate(KTS):
        l, packs, n = _unit_info(nm)
        F, R = FBLK[l], MAXR[l]
        dst = 0
        for bi, off in packs:
            blk = bi * l + bj
            wv = np.array([ws[l][blk][(F - 1) // 2 - d] for d in range(R)],
                          np.float32)
            U = np.zeros((R, R), np.float32)
            for r in range(R):
                U[r, r] = wv[r]
                U[r, r + 1:] = wv[r + 1:] - wv[r:-1]
            out[off:off + R, kt, dst:dst + R] = U.T
            dst += R
    return out


def _pack_weight(w, b, kin_pad, perm=None):
    """(nout, kin) torch-layout weight + bias -> (128, kt, nout) fp16 with
    k rows permuted by `perm` (padded feature -> original), bias NOT
    included (applied via ACT)."""
    w = np.asarray(w, np.float32)
    nout, kin = w.shape
    kts = kin_pad // 128
    out = np.zeros((128, kts, nout), np.float32)
    for kp in range(kin_pad):
        src = perm[kp] if perm is not None else (kp if kp < kin else -1)
        if src >= 0:
            out[kp % 128, kp // 128, :] = w[:, src]
    return np.ascontiguousarray(out.astype(np.float16))


def _pack_w5(w, b):
    """fc5 (1000, 128) + bias -> (128, 2, 1000) fp16; kt1 row0 = bias."""
    w = np.asarray(w, np.float32)
    out = np.zeros((128, 2, NUM_CLASSES), np.float32)
    out[:, 0, :] = w.T
    out[0, 1, :] = np.asarray(b, np.float32)
    return np.ascontiguousarray(out.astype(np.float16))


def _pack_biases(fcs):
    """fc1-fc4 biases -> (128, 29) fp32 columns [fc1 x16, fc2 x8, fc3 x4, fc4]."""
    out = np.zeros((128, 29), np.float32)
    col = 0
    for i, (w, b) in enumerate(fcs[:4]):
        b = np.asarray(b, np.float32)
        for ot in range((FC_DIMS[i + 1] + 127) // 128):
            n0 = ot * 128
            cnt = min(128, FC_DIMS[i + 1] - n0)
            out[0:cnt, col] = b[n0:n0 + cnt]
            col += 1
    assert col == 29
    return out


def _pack_gmbe(gm, be):
    """gamma/beta -> (128, 12) fp32 in padded-permuted order; pads get
    gamma=0 beta=0 (their c is 0 and stays 0)."""
    gm = np.asarray(gm, np.float32)
    be = np.asarray(be, np.float32)
    out = np.zeros((128, 12), np.float32)
    for kp in range(768):
        f = PERM[kp]
        if f >= 0:
            out[kp % 128, kp // 128] = gm[f]
            out[kp % 128, 6 + kp // 128] = be[f]
    return out


# ---------------------------------------------------------------- bass build
def build_program(n_cores=N_CORES, img_per_core=IMG_PER_CORE):
    B = img_per_core
    NG = B // 2
    nc = bacc.Bacc("TRN2", target_bir_lowering=False, debug=False,
                   num_devices=n_cores)

    # ---- DRAM I/O
    x_d = [nc.dram_tensor(f"x{k}", [128, B // 8, 1332], F16,
                          kind="ExternalInput") for k in range(8)]
    at_d = {nm: nc.dram_tensor(f"at_{nm}", [HP, 2, n], F16,
                               kind="ExternalInput")
            for nm, l, pk, n in UNITS}
    mk_d = {nm: nc.dram_tensor(f"mk_{nm}", [n, 2, l, FBLK[l]], F16,
                               kind="ExternalInput")
            for nm, l, pk, n in UNITS}
    uts_d = nc.dram_tensor("uts", [128, 6, 128], F32, kind="ExternalInput")
    wt_d = {}
    for i in range(1, 5):
        kts = FC_DIMS[i - 1] // 128 + (0 if FC_DIMS[i - 1] % 128 == 0 else 1)
        wt_d[i] = nc.dram_tensor(f"w{i}p", [128, kts, FC_DIMS[i]], F16,
                                 kind="ExternalInput")
    wt_d[5] = nc.dram_tensor("w5p", [128, 2, NUM_CLASSES], F16,
                             kind="ExternalInput")
    bi_d = nc.dram_tensor("biasp", [128, 29], F32, kind="ExternalInput")
    gb_d = nc.dram_tensor("gmbe", [128, 12], F32, kind="ExternalInput")
    on_d = nc.dram_tensor("ones", [1, B], F16, kind="ExternalInput")
    out_d = nc.dram_tensor("out", [B, NUM_CLASSES], F32, kind="ExternalOutput")

    AX = mybir.AxisListType
    OP = mybir.AluOpType
    ACT = mybir.ActivationFunctionType

    UINFO = [(nm, l, n) for nm, l, pk, n in UNITS]

    with tile.TileContext(nc) as tc:
        with tc.tile_pool(name="consts", bufs=1) as cp_pool, \
             tc.tile_pool(name="scr", bufs=3) as scr_pool, \
             tc.tile_pool(name="dram", bufs=1, space="DRAM") as dram_pool:

            # ---- weights + late consts prefetch on the vector queue
            w_sb = {}
            for i in range(1, 6):
                kts = wt_d[i].shape[1]
                w_sb[i] = cp_pool.tile([128, kts, FC_DIMS[i]], F16,
                                       name=f"w{i}_sb")
                nc.scalar.dma_start(out=w_sb[i][:], in_=wt_d[i].ap())
            uts_sb = cp_pool.tile([128, 6, 128], F32, name="uts_sb")
            nc.scalar.dma_start(out=uts_sb[:], in_=uts_d.ap())
            bi_sb = cp_pool.tile([128, 29], F32, name="bi_sb")
            nc.scalar.dma_start(out=bi_sb[:], in_=bi_d.ap())
            gb_sb = cp_pool.tile([128, 12], F32, name="gb_sb")
            nc.scalar.dma_start(out=gb_sb[:], in_=gb_d.ap())


            # ---- phase-1 consts on the sync queue (needed immediately)
            at_sb, mk_sb, q_sb = {}, {}, {}
            for nm, l, n in UINFO:
                at_sb[nm] = cp_pool.tile([HP, 2, n], F16, name=f"at_{nm}_sb")
                nc.sync.dma_start(out=at_sb[nm][:], in_=at_d[nm].ap())
                mk_sb[nm] = cp_pool.tile([n, 2, l, FBLK[l]], F16,
                                         name=f"mk_{nm}_sb")
                nc.sync.dma_start(out=mk_sb[nm][:], in_=mk_d[nm].ap())
                q_sb[nm] = cp_pool.tile([n, B, l], F32, name=f"q_{nm}")

            # ---- whole x shard SBUF-resident, loaded from eight 128-row
            # dram blobs. The DGE splits a DMA's descriptor list EVENLY
            # across engines, so 128-descriptor DMAs use all 16 engines
            # (8 each) while 111-descriptor ones collapse to 3 (111=3*37):
            # pad the partition dim to 128 (rows 111-127 are zeros).
            xg = cp_pool.tile([128, B, 1332], F16, name="xg_all")
            for k in range(8):
                i0 = k * (B // 8)
                eng = (nc.sync, nc.scalar)[k % 2]
                eng.dma_start(out=xg[0:128, i0:i0 + B // 8, :],
                              in_=x_d[k].ap())

            # ---- phase 1: segment-reduce to Q from the resident x tile
            with tc.tile_pool(name="zp", bufs=2, space="PSUM") as zp_pool:
                for g in range(NG):
                    for nm, l, n in UINFO:
                        F = FBLK[l]
                        ch = l - 1
                        zp = zp_pool.tile([n, 2, l, F], F32, tag=f"z{nm}")
                        for par in (0, 1):
                            # rhs streams (img, col) = (img, bj, jj) in order
                            nc.tensor.matmul(
                                zp[:], at_sb[nm][0:HP, par, :],
                                xg[0:HP, 2 * g:2 * g + 2,
                                   par * 666 + ch:par * 666 + ch + 664:3],
                                start=(par == 0), stop=(par == 1))
                        sc = scr_pool.tile([n, 2, l, F], F16, tag=f"sc{nm}")
                        nc.scalar.copy(sc[:], zp[:])
                        mm = scr_pool.tile([n, 2, l, F], F16, tag=f"mm{nm}")
                        me = (nc.gpsimd if (nm == "u2" and g >= 8)
                              else nc.vector)
                        me.tensor_tensor(mm[:], sc[:], mk_sb[nm][:], OP.mult)
                        nc.vector.tensor_reduce(
                            q_sb[nm][0:n, 2 * g:2 * g + 2, 0:l],
                            mm[:], AX.X, OP.add)

            # ---- phase 2: c_kt = UTS_kt^T @ q  (padded 768-feature space)
            c_ps = []
            stats = cp_pool.tile([128, 12], F32, name="stats")
            sqscr = cp_pool.tile([128, B], F32, name="sqscr")
            c_sb = cp_pool.tile([128, 6, B], F32, name="c_sb")
            with tc.tile_pool(name="cps", bufs=1, space="PSUM") as cps_pool:
                for kt, (nm, bj) in enumerate(KTS):
                    l, _, n = _unit_info(nm)
                    cps = cps_pool.tile([128, B], F32, tag=f"c{kt}")
                    nc.tensor.matmul(
                        cps[:], uts_sb[0:n, kt, :], q_sb[nm][0:n, :, bj],
                        start=True, stop=True)
                    c_ps.append(cps)
                    # ---- phase 3: batch-norm partial stats per k-tile
                    nc.scalar.copy(c_sb[0:128, kt, :], cps[:])
                    nc.vector.tensor_reduce(
                        stats[0:128, kt:kt + 1], cps[:], AX.X, OP.add)
                    nc.vector.tensor_tensor(
                        sqscr[:], c_sb[0:128, kt, :], c_sb[0:128, kt, :],
                        OP.mult)
                    nc.vector.tensor_reduce(
                        stats[0:128, 6 + kt:7 + kt], sqscr[:], AX.X, OP.add)

                # ---- AllReduce of stats
                cc_in = dram_pool.tile([128, 12], F32)
                cc_out = dram_pool.tile(
                    [128, 12], F32,
                    addr_space="Shared" if n_cores > 4 else "Local")
                nc.scalar.dma_start(out=cc_in[:], in_=stats[:])
                nc.gpsimd.collective_compute(
                    "AllReduce", OP.add,
                    replica_groups=[list(range(n_cores))],
                    ins=[cc_in[:].opt()], outs=[cc_out[:].opt()])
                statg = cp_pool.tile([128, 12], F32, name="statg")
                nc.scalar.dma_start(out=statg[:], in_=cc_out[:])

            # ---- phase 4: d = gamma * rsqrt(var + eps); cn = d*c + e
            nb = float(n_cores * B)
            bnd = cp_pool.tile([128, 6], F32, name="bnd")
            bne = cp_pool.tile([128, 6], F32, name="bne")
            mu = cp_pool.tile([128, 6], F32, name="mu")
            vtmp = cp_pool.tile([128, 6], F32, name="vtmp")
            nc.scalar.mul(mu[:], statg[0:128, 0:6], 1.0 / nb)
            nc.scalar.mul(vtmp[:], statg[0:128, 6:12], 1.0 / nb)
            nc.vector.tensor_tensor(bnd[:], mu[:], mu[:], OP.mult)
            nc.vector.tensor_tensor(vtmp[:], vtmp[:], bnd[:], OP.subtract)
            eps = cp_pool.tile([128, 1], F32, name="eps")
            nc.vector.memset(eps[:], 1e-5)
            nc.scalar.activation(vtmp[:], vtmp[:], ACT.Sqrt, bias=eps[:])
            nc.vector.reciprocal(vtmp[:], vtmp[:])
            nc.vector.tensor_tensor(bnd[:], gb_sb[0:128, 0:6], vtmp[:], OP.mult)
            nc.vector.tensor_tensor(vtmp[:], mu[:], bnd[:], OP.mult)
            nc.vector.tensor_tensor(bne[:], gb_sb[0:128, 6:12], vtmp[:],
                                    OP.subtract)
            cn_sb = cp_pool.tile([128, 6, B], F16, name="cn_sb")
            for kt in range(6):
                nc.vector.tensor_scalar(
                    out=cn_sb[0:128, kt, :], in0=c_sb[0:128, kt, :],
                    scalar1=bnd[0:128, kt:kt + 1],
                    scalar2=bne[0:128, kt:kt + 1],
                    op0=OP.mult, op1=OP.add)

            # ---- phase 5: feature-major MLP, stationary SBUF weights
            h = cn_sb
            bcol = 0
            with tc.tile_pool(name="mps", bufs=4, space="PSUM") as mps_pool:
                for i in range(1, 5):
                    kts = w_sb[i].shape[1]
                    nout = FC_DIMS[i]
                    nots = (nout + 127) // 128
                    extra = 1 if i == 4 else 0   # fc4 output carries ones kt
                    hn = cp_pool.tile([128, nots + extra, B], F16,
                                      name=f"h{i}")
                    for ot in range(nots):
                        n0 = ot * 128
                        cnt = min(128, nout - n0)
                        ps = mps_pool.tile([cnt, B], F32, tag="mp",
                                           name=f"mp{i}_{ot}")
                        for kt in range(kts):
                            nc.tensor.matmul(
                                ps[:], w_sb[i][0:128, kt, n0:n0 + cnt],
                                h[0:128, kt, :],
                                start=(kt == 0), stop=(kt == kts - 1))
                        nc.scalar.activation(
                            hn[0:cnt, ot, :], ps[:], ACT.Relu,
                            bias=bi_sb[0:cnt, bcol + ot:bcol + ot + 1])
                    bcol += nots
                    h = hn
                # fc4 ones k-tile for the fc5 bias row
                nc.sync.dma_start(out=h[0:1, 1, :], in_=on_d.ap())

                # fc5 batch-major: h4 k-tiles stationary, W5 moving
                out_sb = cp_pool.tile([B, NUM_CLASSES], F32, name="out_sb")
                for half in range(2):
                    n0, n1 = half * 500, 500 + half * 500
                    ps = mps_pool.tile([B, 500], F32, tag="mp",
                                       name=f"mp5_{half}")
                    for kt, kp in ((0, 128), (1, 1)):
                        nc.tensor.matmul(
                            ps[:], h[0:kp, kt, :], w_sb[5][0:kp, kt, n0:n1],
                            start=(kt == 0), stop=(kt == 1))
                    nc.scalar.copy(out_sb[0:B, n0:n1], ps[:])
            nc.sync.dma_start(out=out_d.ap(), in_=out_sb[:])

    nc.compile()
    return nc


# ------------------------------------------------------------------- runtime
_CACHE = {}


def _get_program():
    key = (N_CORES, IMG_PER_CORE)
    if key not in _CACHE:
        _CACHE[key] = build_program(*key)
    return _CACHE[key]


def _host_consts(w1, w2, w3, bn_gamma, bn_beta, fcs):
    consts = {}
    for nm, l, pk, n in UNITS:
        consts[f"at_{nm}"] = _unit_A(nm)
        consts[f"mk_{nm}"] = _unit_M(nm)
    consts["uts"] = _build_UTS(w1, w2, w3)
    consts["w1p"] = _pack_weight(fcs[0][0], fcs[0][1], 768, perm=PERM)
    consts["w2p"] = _pack_weight(fcs[1][0], fcs[1][1], 2048)
    consts["w3p"] = _pack_weight(fcs[2][0], fcs[2][1], 1024)
    consts["w4p"] = _pack_weight(fcs[3][0], fcs[3][1], 512)
    consts["w5p"] = _pack_w5(fcs[4][0], fcs[4][1])
    consts["biasp"] = _pack_biases(fcs)
    consts["gmbe"] = _pack_gmbe(bn_gamma, bn_beta)
    consts["ones"] = np.ones((1, IMG_PER_CORE), np.float16)
    return consts


def kernel(x, w1, w2, w3, bn_gamma, bn_beta,
           fc1_w, fc1_b, fc2_w, fc2_b, fc3_w, fc3_b, fc4_w, fc4_b,
           fc5_w, fc5_b):
    from concourse.bass_utils import run_bass_kernel_spmd

    nc = _get_program()
    consts = _host_consts(
        w1, w2, w3, bn_gamma, bn_beta,
        [(fc1_w, fc1_b), (fc2_w, fc2_b), (fc3_w, fc3_b), (fc4_w, fc4_b),
         (fc5_w, fc5_b)])
    xh = np.asarray(x, np.float32).astype(np.float16)
    xh = xh.reshape(N_CORES, IMG_PER_CORE, HP, 1332)
    q8 = IMG_PER_CORE // 8
    in_maps = []
    for s in range(N_CORES):
        m = dict(consts)
        xt = np.zeros((128, IMG_PER_CORE, 1332), np.float16)
        xt[0:HP] = xh[s].transpose(1, 0, 2)
        for k in range(8):
            m[f"x{k}"] = np.ascontiguousarray(xt[:, k * q8:(k + 1) * q8, :])
        in_maps.append(m)

    trace = bool(int(os.environ.get("BASSDFT_TRACE", "0")))
    if trace:
        _install_ntff_hook()
    res = run_bass_kernel_spmd(nc, in_maps, core_ids=list(range(N_CORES)),
                               trace=trace)
    if trace:
        kernel.last_exec_time_ns = res.exec_time_ns
        kernel.last_results = res
    return np.concatenate([res.results[s]["out"] for s in range(N_CORES)],
                          axis=0)


def _install_ntff_hook():
    """Register the axon NTFF profiling hook (antenv.axon_hooks is absent in
    this image) and disable the share-bucket artifact upload."""
    try:
        from antenv import axon_hooks  # noqa: F401
        return
    except ImportError:
        pass
    try:
        from trn_agent_boot.trn_boot import _ntff_profile_via_ctypes
    except ImportError:
        return
    import antenv
    import concourse.bass_utils as bu
    mod = types.ModuleType("antenv.axon_hooks")
    hook = [_ntff_profile_via_ctypes("/opt/axon/libaxon_pjrt.so")]
    mod.get_axon_ntff_profile_hook = lambda: hook[0]
    mod.set_axon_ntff_profile_hook = lambda h: hook.__setitem__(0, h)
    sys.modules["antenv.axon_hooks"] = mod
    antenv.axon_hooks = mod
    bu.upload_artifacts = lambda tmpdir: tmpdir


# revision 20
# speedup vs baseline: 1.7771x; 1.2440x over previous
"""Trainium2 Bass kernel for nn_DFTModel (segment_reduce), v2.

Math: for each level l in {1,2,3} the reference multiplies channel l-1 by a
block-tiled radial ("square ring") filter and sums nested square rectangles.
Nested-square sums of the raw image Q[r] are computed as
    Q = rowsum(M o (A~ @ X))       (PE matmul with 0/1 interval operator A~,
                                    mask-multiply + axis reduce)
and the filter weights fold into a tiny per-block upper-triangular transform
    c = U @ Q
so the 222x222 filter product is never materialized.

v2 layout/schedule changes vs v1 (which was serialized on the Sync queue's
DMA issue rate at ~94 GB/s effective):
  * x is converted to float16 on the host and stored 2 rows per partition
    (111 partitions x 1332 elems, one contiguous 2664 B descriptor per
    partition) - half the bytes and half the DMA descriptors. The row-parity
    split becomes two accumulating matmuls per unit with parity-split A~
    operators.
  * image DMAs are spread across two queues (sync early, gpsimd late - the
    gpsimd queue sits behind the startup all-core barrier); MLP weights are
    prefetched in fp16 on the vector queue at t=0 and stay SBUF-resident.
  * level 3 is repacked into ONE 111-row unit (row offsets 0/37/74). All
    engine APs start at partition base 0, so no DMA scatter is needed
    anywhere: phase 2 is six stacked-operator matmuls (one per 128-feature
    tile of a padded 768-feature space), and fc1 weights / bn params are
    host-permuted to match.
  * phase-1 masked reduce is split across engines: ACT casts PSUM z to fp16
    in SBUF, mask-multiplies run on DVE (2x fp16 mode) for early groups and
    on GpSimd for late groups, DVE does the axis reduces into fp32 q.
  * the MLP runs feature-major with stationary fp16 weights (out[n, img]),
    biases applied by the ACT engine per partition - no transposes, no ones
    rows except fc5, whose bias rides as an extra weight k-tile against an
    all-ones activation row.
BatchNorm batch statistics are exchanged with one 6 KB AllReduce.
"""

import os
import sys
import types

import numpy as np

for _p in ("/opt/trn_rl_repo",):
    if _p not in sys.path and os.path.isdir(_p):
        sys.path.insert(0, _p)

import concourse.bacc as bacc
import concourse.bass as bass
import concourse.mybir as mybir
import concourse.tile as tile

F32 = mybir.dt.float32
F16 = mybir.dt.float16

IMAGE = 222
HP = 111                      # partitions for the 2-rows-per-partition layout
N_CORES = 8
IMG_PER_CORE = 32
NUM_CLASSES = 1000
FC_DIMS = [768, 2048, 1024, 512, 128, 1000]   # 768 = padded feature count


def _get_dim(level):
    d = IMAGE / level
    if d % 2 == 1:
        d += 1
    return int(d // 2)


MAXR = {l: _get_dim(l) for l in (1, 2, 3)}          # 111, 56, 37
FBLK = {l: IMAGE // l for l in (1, 2, 3)}           # 222, 111, 74
NUM_COEFF = sum(l * l * MAXR[l] for l in (1, 2, 3))  # 668

# units: (name, level, [(bi, row_offset)...], n_rows)
UNITS = [
    ("u1", 1, [(0, 0)], 111),
    ("u2", 2, [(0, 0), (1, 64)], 120),
    ("u3", 3, [(0, 0), (1, 37), (2, 74)], 111),
]

# phase-2 k-tiles of the padded 768-feature space: (q-unit, bj)
KTS = [("u1", 0), ("u2", 0), ("u2", 1), ("u3", 0), ("u3", 1), ("u3", 2)]


def _unit_info(name):
    for nm, lv, pk, nr in UNITS:
        if nm == name:
            return lv, pk, nr
    raise KeyError(name)


def _orig_block_off(l, bi, bj):
    base = {1: 0, 2: 111, 3: 335}[l]
    return base + (bi * l + bj) * MAXR[l]


def _padded_map():
    """perm[kt*128 + p] = original feature index, or -1 for padding."""
    perm = np.full(768, -1, np.int64)
    for kt, (nm, bj) in enumerate(KTS):
        l, packs, n = _unit_info(nm)
        R = MAXR[l]
        dst = 0
        for bi, off in packs:
            f0 = _orig_block_off(l, bi, bj)
            perm[kt * 128 + dst:kt * 128 + dst + R] = np.arange(f0, f0 + R)
            dst += R
    return perm


PERM = _padded_map()


# ---------------------------------------------------------------- host consts
def _unit_A(name):
    """Parity-split 0/1 interval operator: (111, 2, n_rows) fp16.
    at[p, par, off+r] = 1 iff (2p+par) is in [bi*F + r, (bi+1)*F - r)."""
    l, packs, n = _unit_info(name)
    F, R = FBLK[l], MAXR[l]
    A = np.zeros((n, IMAGE), np.float32)
    for bi, off in packs:
        for r in range(R):
            A[off + r, bi * F + r:(bi + 1) * F - r] = 1.0
    A = A.T.reshape(HP, 2, n)
    return np.ascontiguousarray(A.astype(np.float16))


def _unit_M(name):
    """Mask (n_rows, 2 imgs, l, F) fp16, zero on pad rows."""
    l, packs, n = _unit_info(name)
    F, R = FBLK[l], MAXR[l]
    M = np.zeros((n, F), np.float32)
    for _, off in packs:
        for r in range(R):
            M[off + r, r:F - r] = 1.0
    out = np.broadcast_to(M[:, None, None, :], (n, 2, l, F))
    return np.ascontiguousarray(out.astype(np.float16))


def _build_UTS(w1, w2, w3):
    """Stacked phase-2 operators (128, 6, 128) fp32.
    uts[r, kt, d] so that c_kt[f, b] = sum_r uts[r, kt, f] * q_unit[r, b]."""
    ws = {1: np.asarray(w1, np.float32).reshape(1, -1),
          2: np.asarray(w2, np.float32),
          3: np.asarray(w3, np.float32)}
    out = np.zeros((128, 6, 128), np.float32)
    for kt, (nm, bj) in enumerate(KTS):
        l, packs, n = _unit_info(nm)
        F, R = FBLK[l], MAXR[l]
        dst = 0
        for bi, off in packs:
            blk = bi * l + bj
            wv = np.array([ws[l][blk][(F - 1) // 2 - d] for d in range(R)],
                          np.float32)
            U = np.zeros((R, R), np.float32)
            for r in range(R):
                U[r, r] = wv[r]
                U[r, r + 1:] = wv[r + 1:] - wv[r:-1]
            out[off:off + R, kt, dst:dst + R] = U.T
            dst += R
    return out


def _pack_weight(w, b, kin_pad, perm=None):
    """(nout, kin) torch-layout weight + bias -> (128, kt, nout) fp16 with
    k rows permuted by `perm` (padded feature -> original), bias NOT
    included (applied via ACT)."""
    w = np.asarray(w, np.float32)
    nout, kin = w.shape
    kts = kin_pad // 128
    out = np.zeros((128, kts, nout), np.float32)
    for kp in range(kin_pad):
        src = perm[kp] if perm is not None else (kp if kp < kin else -1)
        if src >= 0:
            out[kp % 128, kp // 128, :] = w[:, src]
    return np.ascontiguousarray(out.astype(np.float16))


def _pack_w5(w, b):
    """fc5 (1000, 128) + bias -> (128, 2, 1000) fp16; kt1 row0 = bias."""
    w = np.asarray(w, np.float32)
    out = np.zeros((128, 2, NUM_CLASSES), np.float32)
    out[:, 0, :] = w.T
    out[0, 1, :] = np.asarray(b, np.float32)
    return np.ascontiguousarray(out.astype(np.float16))


def _pack_biases(fcs):
    """fc1-fc4 biases -> (128, 29) fp32 columns [fc1 x16, fc2 x8, fc3 x4, fc4]."""
    out = np.zeros((128, 29), np.float32)
    col = 0
    for i, (w, b) in enumerate(fcs[:4]):
        b = np.asarray(b, np.float32)
        for ot in range((FC_DIMS[i + 1] + 127) // 128):
            n0 = ot * 128
            cnt = min(128, FC_DIMS[i + 1] - n0)
            out[0:cnt, col] = b[n0:n0 + cnt]
            col += 1
    assert col == 29
    return out


def _pack_gmbe(gm, be):
    """gamma/beta -> (128, 12) fp32 in padded-permuted order; pads get
    gamma=0 beta=0 (their c is 0 and stays 0)."""
    gm = np.asarray(gm, np.float32)
    be = np.asarray(be, np.float32)
    out = np.zeros((128, 12), np.float32)
    for kp in range(768):
        f = PERM[kp]
        if f >= 0:
            out[kp % 128, kp // 128] = gm[f]
            out[kp % 128, 6 + kp // 128] = be[f]
    return out


# ---------------------------------------------------------------- bass build
def build_program(n_cores=N_CORES, img_per_core=IMG_PER_CORE):
    B = img_per_core
    NG = B // 2
    nc = bacc.Bacc("TRN2", target_bir_lowering=False, debug=False,
                   num_devices=n_cores)

    # ---- DRAM I/O
    x_d = [nc.dram_tensor(f"x{k}", [128, B // 8, 1332], F16,
                          kind="ExternalInput") for k in range(8)]
    at_d = {nm: nc.dram_tensor(f"at_{nm}", [HP, 2, n], F16,
                               kind="ExternalInput")
            for nm, l, pk, n in UNITS}
    mk_d = {nm: nc.dram_tensor(f"mk_{nm}", [n, 2, l, FBLK[l]], F16,
                               kind="ExternalInput")
            for nm, l, pk, n in UNITS}
    uts_d = nc.dram_tensor("uts", [128, 6, 128], F32, kind="ExternalInput")
    wt_d = {}
    for i in range(1, 5):
        kts = FC_DIMS[i - 1] // 128 + (0 if FC_DIMS[i - 1] % 128 == 0 else 1)
        wt_d[i] = nc.dram_tensor(f"w{i}p", [128, kts, FC_DIMS[i]], F16,
                                 kind="ExternalInput")
    wt_d[5] = nc.dram_tensor("w5p", [128, 2, NUM_CLASSES], F16,
                             kind="ExternalInput")
    bi_d = nc.dram_tensor("biasp", [128, 29], F32, kind="ExternalInput")
    gb_d = nc.dram_tensor("gmbe", [128, 12], F32, kind="ExternalInput")
    on_d = nc.dram_tensor("ones", [1, B], F16, kind="ExternalInput")
    out_d = nc.dram_tensor("out", [B, NUM_CLASSES], F32, kind="ExternalOutput")

    AX = mybir.AxisListType
    OP = mybir.AluOpType
    ACT = mybir.ActivationFunctionType

    UINFO = [(nm, l, n) for nm, l, pk, n in UNITS]

    with tile.TileContext(nc) as tc:
        with tc.tile_pool(name="consts", bufs=1) as cp_pool, \
             tc.tile_pool(name="scr", bufs=3) as scr_pool, \
             tc.tile_pool(name="dram", bufs=1, space="DRAM") as dram_pool:


            # ---- phase-1 consts on the sync queue (needed immediately)
            at_sb, mk_sb, q_sb = {}, {}, {}
            for nm, l, n in UINFO:
                at_sb[nm] = cp_pool.tile([HP, 2, n], F16, name=f"at_{nm}_sb")
                nc.sync.dma_start(out=at_sb[nm][:], in_=at_d[nm].ap())
                mk_sb[nm] = cp_pool.tile([n, 2, l, FBLK[l]], F16,
                                         name=f"mk_{nm}_sb")
                nc.sync.dma_start(out=mk_sb[nm][:], in_=mk_d[nm].ap())
                q_sb[nm] = cp_pool.tile([n, B, l], F32, name=f"q_{nm}")

            # ---- whole x shard SBUF-resident, loaded from eight 128-row
            # dram blobs. The DGE splits a DMA's descriptor list EVENLY
            # across engines, so 128-descriptor DMAs use all 16 engines
            # (8 each) while 111-descriptor ones collapse to 3 (111=3*37):
            # pad the partition dim to 128 (rows 111-127 are zeros).
            xg = cp_pool.tile([128, B, 1332], F16, name="xg_all")
            for k in range(8):
                i0 = k * (B // 8)
                nc.sync.dma_start(out=xg[0:128, i0:i0 + B // 8, :],
                                  in_=x_d[k].ap())

            # ---- weights + late consts AFTER x in sync-queue order so the
            # image stream gets the full DMA-engine pool first
            w_sb = {}
            for i in range(1, 6):
                kts = wt_d[i].shape[1]
                w_sb[i] = cp_pool.tile([128, kts, FC_DIMS[i]], F16,
                                       name=f"w{i}_sb")
                nc.sync.dma_start(out=w_sb[i][:], in_=wt_d[i].ap())
            uts_sb = cp_pool.tile([128, 6, 128], F32, name="uts_sb")
            nc.sync.dma_start(out=uts_sb[:], in_=uts_d.ap())
            bi_sb = cp_pool.tile([128, 29], F32, name="bi_sb")
            nc.sync.dma_start(out=bi_sb[:], in_=bi_d.ap())
            gb_sb = cp_pool.tile([128, 12], F32, name="gb_sb")
            nc.sync.dma_start(out=gb_sb[:], in_=gb_d.ap())

            # ---- phase 1: segment-reduce to Q from the resident x tile
            with tc.tile_pool(name="zp", bufs=2, space="PSUM") as zp_pool:
                for g in range(NG):
                    for nm, l, n in UINFO:
                        F = FBLK[l]
                        ch = l - 1
                        zp = zp_pool.tile([n, 2, l, F], F32, tag=f"z{nm}")
                        for par in (0, 1):
                            # rhs streams (img, col) = (img, bj, jj) in order
                            nc.tensor.matmul(
                                zp[:], at_sb[nm][0:HP, par, :],
                                xg[0:HP, 2 * g:2 * g + 2,
                                   par * 666 + ch:par * 666 + ch + 664:3],
                                start=(par == 0), stop=(par == 1))
                        sc = scr_pool.tile([n, 2, l, F], F16, tag=f"sc{nm}")
                        nc.scalar.copy(sc[:], zp[:])
                        mm = scr_pool.tile([n, 2, l, F], F16, tag=f"mm{nm}")
                        me = (nc.gpsimd if (nm == "u2" and g >= 8)
                              else nc.vector)
                        me.tensor_tensor(mm[:], sc[:], mk_sb[nm][:], OP.mult)
                        nc.vector.tensor_reduce(
                            q_sb[nm][0:n, 2 * g:2 * g + 2, 0:l],
                            mm[:], AX.X, OP.add)

            # ---- phase 2: c_kt = UTS_kt^T @ q  (padded 768-feature space)
            c_ps = []
            stats = cp_pool.tile([128, 12], F32, name="stats")
            sqscr = cp_pool.tile([128, B], F32, name="sqscr")
            c_sb = cp_pool.tile([128, 6, B], F32, name="c_sb")
            with tc.tile_pool(name="cps", bufs=1, space="PSUM") as cps_pool:
                for kt, (nm, bj) in enumerate(KTS):
                    l, _, n = _unit_info(nm)
                    cps = cps_pool.tile([128, B], F32, tag=f"c{kt}")
                    nc.tensor.matmul(
                        cps[:], uts_sb[0:n, kt, :], q_sb[nm][0:n, :, bj],
                        start=True, stop=True)
                    c_ps.append(cps)
                    # ---- phase 3: batch-norm partial stats per k-tile
                    nc.scalar.copy(c_sb[0:128, kt, :], cps[:])
                    nc.vector.tensor_reduce(
                        stats[0:128, kt:kt + 1], cps[:], AX.X, OP.add)
                    nc.vector.tensor_tensor(
                        sqscr[:], c_sb[0:128, kt, :], c_sb[0:128, kt, :],
                        OP.mult)
                    nc.vector.tensor_reduce(
                        stats[0:128, 6 + kt:7 + kt], sqscr[:], AX.X, OP.add)

                # ---- AllReduce of stats
                cc_in = dram_pool.tile([128, 12], F32)
                cc_out = dram_pool.tile(
                    [128, 12], F32,
                    addr_space="Shared" if n_cores > 4 else "Local")
                nc.scalar.dma_start(out=cc_in[:], in_=stats[:])
                nc.gpsimd.collective_compute(
                    "AllReduce", OP.add,
                    replica_groups=[list(range(n_cores))],
                    ins=[cc_in[:].opt()], outs=[cc_out[:].opt()])
                statg = cp_pool.tile([128, 12], F32, name="statg")
                nc.scalar.dma_start(out=statg[:], in_=cc_out[:])

            # ---- phase 4: d = gamma * rsqrt(var + eps); cn = d*c + e
            nb = float(n_cores * B)
            bnd = cp_pool.tile([128, 6], F32, name="bnd")
            bne = cp_pool.tile([128, 6], F32, name="bne")
            mu = cp_pool.tile([128, 6], F32, name="mu")
            vtmp = cp_pool.tile([128, 6], F32, name="vtmp")
            nc.scalar.mul(mu[:], statg[0:128, 0:6], 1.0 / nb)
            nc.scalar.mul(vtmp[:], statg[0:128, 6:12], 1.0 / nb)
            nc.vector.tensor_tensor(bnd[:], mu[:], mu[:], OP.mult)
            nc.vector.tensor_tensor(vtmp[:], vtmp[:], bnd[:], OP.subtract)
            eps = cp_pool.tile([128, 1], F32, name="eps")
            nc.vector.memset(eps[:], 1e-5)
            nc.scalar.activation(vtmp[:], vtmp[:], ACT.Sqrt, bias=eps[:])
            nc.vector.reciprocal(vtmp[:], vtmp[:])
            nc.vector.tensor_tensor(bnd[:], gb_sb[0:128, 0:6], vtmp[:], OP.mult)
            nc.vector.tensor_tensor(vtmp[:], mu[:], bnd[:], OP.mult)
            nc.vector.tensor_tensor(bne[:], gb_sb[0:128, 6:12], vtmp[:],
                                    OP.subtract)
            cn_sb = cp_pool.tile([128, 6, B], F16, name="cn_sb")
            for kt in range(6):
                nc.vector.tensor_scalar(
                    out=cn_sb[0:128, kt, :], in0=c_sb[0:128, kt, :],
                    scalar1=bnd[0:128, kt:kt + 1],
                    scalar2=bne[0:128, kt:kt + 1],
                    op0=OP.mult, op1=OP.add)

            # ---- phase 5: feature-major MLP, stationary SBUF weights
            h = cn_sb
            bcol = 0
            with tc.tile_pool(name="mps", bufs=4, space="PSUM") as mps_pool:
                for i in range(1, 5):
                    kts = w_sb[i].shape[1]
                    nout = FC_DIMS[i]
                    nots = (nout + 127) // 128
                    extra = 1 if i == 4 else 0   # fc4 output carries ones kt
                    hn = cp_pool.tile([128, nots + extra, B], F16,
                                      name=f"h{i}")
                    for ot in range(nots):
                        n0 = ot * 128
                        cnt = min(128, nout - n0)
                        ps = mps_pool.tile([cnt, B], F32, tag="mp",
                                           name=f"mp{i}_{ot}")
                        for kt in range(kts):
                            nc.tensor.matmul(
                                ps[:], w_sb[i][0:128, kt, n0:n0 + cnt],
                                h[0:128, kt, :],
                                start=(kt == 0), stop=(kt == kts - 1))
                        nc.scalar.activation(
                            hn[0:cnt, ot, :], ps[:], ACT.Relu,
                            bias=bi_sb[0:cnt, bcol + ot:bcol + ot + 1])
                    bcol += nots
                    h = hn
                # fc4 ones k-tile for the fc5 bias row
                nc.sync.dma_start(out=h[0:1, 1, :], in_=on_d.ap())

                # fc5 batch-major: h4 k-tiles stationary, W5 moving
                out_sb = cp_pool.tile([B, NUM_CLASSES], F32, name="out_sb")
                for half in range(2):
                    n0, n1 = half * 500, 500 + half * 500
                    ps = mps_pool.tile([B, 500], F32, tag="mp",
                                       name=f"mp5_{half}")
                    for kt, kp in ((0, 128), (1, 1)):
                        nc.tensor.matmul(
                            ps[:], h[0:kp, kt, :], w_sb[5][0:kp, kt, n0:n1],
                            start=(kt == 0), stop=(kt == 1))
                    nc.scalar.copy(out_sb[0:B, n0:n1], ps[:])
            nc.sync.dma_start(out=out_d.ap(), in_=out_sb[:])

    nc.compile()
    return nc


# ------------------------------------------------------------------- runtime
_CACHE = {}


def _get_program():
    key = (N_CORES, IMG_PER_CORE)
    if key not in _CACHE:
        _CACHE[key] = build_program(*key)
    return _CACHE[key]


def _host_consts(w1, w2, w3, bn_gamma, bn_beta, fcs):
    consts = {}
    for nm, l, pk, n in UNITS:
        consts[f"at_{nm}"] = _unit_A(nm)
        consts[f"mk_{nm}"] = _unit_M(nm)
    consts["uts"] = _build_UTS(w1, w2, w3)
    consts["w1p"] = _pack_weight(fcs[0][0], fcs[0][1], 768, perm=PERM)
    consts["w2p"] = _pack_weight(fcs[1][0], fcs[1][1], 2048)
    consts["w3p"] = _pack_weight(fcs[2][0], fcs[2][1], 1024)
    consts["w4p"] = _pack_weight(fcs[3][0], fcs[3][1], 512)
    consts["w5p"] = _pack_w5(fcs[4][0], fcs[4][1])
    consts["biasp"] = _pack_biases(fcs)
    consts["gmbe"] = _pack_gmbe(bn_gamma, bn_beta)
    consts["ones"] = np.ones((1, IMG_PER_CORE), np.float16)
    return consts


def kernel(x, w1, w2, w3, bn_gamma, bn_beta,
           fc1_w, fc1_b, fc2_w, fc2_b, fc3_w, fc3_b, fc4_w, fc4_b,
           fc5_w, fc5_b):
    from concourse.bass_utils import run_bass_kernel_spmd

    nc = _get_program()
    consts = _host_consts(
        w1, w2, w3, bn_gamma, bn_beta,
        [(fc1_w, fc1_b), (fc2_w, fc2_b), (fc3_w, fc3_b), (fc4_w, fc4_b),
         (fc5_w, fc5_b)])
    xh = np.asarray(x, np.float32).astype(np.float16)
    xh = xh.reshape(N_CORES, IMG_PER_CORE, HP, 1332)
    q8 = IMG_PER_CORE // 8
    in_maps = []
    for s in range(N_CORES):
        m = dict(consts)
        xt = np.zeros((128, IMG_PER_CORE, 1332), np.float16)
        xt[0:HP] = xh[s].transpose(1, 0, 2)
        for k in range(8):
            m[f"x{k}"] = np.ascontiguousarray(xt[:, k * q8:(k + 1) * q8, :])
        in_maps.append(m)

    trace = bool(int(os.environ.get("BASSDFT_TRACE", "0")))
    if trace:
        _install_ntff_hook()
    res = run_bass_kernel_spmd(nc, in_maps, core_ids=list(range(N_CORES)),
                               trace=trace)
    if trace:
        kernel.last_exec_time_ns = res.exec_time_ns
        kernel.last_results = res
    return np.concatenate([res.results[s]["out"] for s in range(N_CORES)],
                          axis=0)


def _install_ntff_hook():
    """Register the axon NTFF profiling hook (antenv.axon_hooks is absent in
    this image) and disable the share-bucket artifact upload."""
    try:
        from antenv import axon_hooks  # noqa: F401
        return
    except ImportError:
        pass
    try:
        from trn_agent_boot.trn_boot import _ntff_profile_via_ctypes
    except ImportError:
        return
    import antenv
    import concourse.bass_utils as bu
    mod = types.ModuleType("antenv.axon_hooks")
    hook = [_ntff_profile_via_ctypes("/opt/axon/libaxon_pjrt.so")]
    mod.get_axon_ntff_profile_hook = lambda: hook[0]
    mod.set_axon_ntff_profile_hook = lambda h: hook.__setitem__(0, h)
    sys.modules["antenv.axon_hooks"] = mod
    antenv.axon_hooks = mod
    bu.upload_artifacts = lambda tmpdir: tmpdir
